# revision 1
# baseline (speedup 1.0000x reference)
"""Friend-attention pooling kernel for Trainium2 (8 NeuronCores, SPMD).

Problem (hardcoded shapes):
    friend_diff_x [16384, 50, 128] f32, self_x [256, 128] f32,
    friend_diff_src_mask [16384, 50] bool, friend_num_src == 64.
    out[b, f, :] = sum_l softmax_l(X[n] @ s[b])[l] * mask[n, l] * X[n, l, :]
    with n = b*64 + f.

Strategy: data-parallel over users across 8 cores (2048 friend rows / core,
16 blocks of 128 friends = 2 users each). Host pre-packs friend_diff_x into
TWO fp16 copies so BOTH contractions run on TensorE (DVE fp32 is 1
elem/lane/cycle and would be the bottleneck):
  - xt: per-friend transposed [D=128 part, (friend, L) free] -> phase-1
        score matmuls (contract D). One-hot stationary columns make the 16
        chunk matmuls of a block accumulate into disjoint rows of one
        [16, 400] PSUM tile (PSUM cannot be DMA'd; dense rows allow a
        single ACT copy out + one scatter DMA to [128 friends, 50]).
  - xn: mask-premultiplied natural layout, friend-pairs stacked
        [(fip,L)->114 part (50 + 14 zero pad + 50), (pair, D) free]
        -> phase-2 pooling matmuls (contract K=114 incl. zero pad),
        X-pair stationary, block-diagonal softmax-weight columns moving;
        output accumulates TRANSPOSED [D, friend] densely in PSUM
        (host un-transposes). Mask folding is exact in fp16.
Softmax on [128 friends, 50]: ACT exp with fused accum denominator, DVE
reciprocal + tensor_scalar normalize, PE transpose to [L, friend].

Raw bass (manual semaphores): this walrus allows only ONE sync-wait per
DMA instruction, which Tile's pool-rotation wait sets exceed. All cross-
engine waits are standalone wait_ge instructions; DMAs carry only their
completion increment. PE stream is software-pipelined (phase1 of block b
issues before transpose/phase2 of block b-1) to hide softmax latency.
"""

from contextlib import ExitStack

import numpy as np

import concourse.bass as bass
from concourse import mybir
from concourse.bass_utils import run_bass_kernel_spmd

B = 256          # users
FPER = 64        # friends per user
L = 50           # history length (softmax axis)
D = 128          # embed dim
N = B * FPER     # 16384 friend rows
NCORES = 8
FCORE = N // NCORES      # 2048 friend rows per core
BF = 128                 # friends per block (= 2 users)
NPAIR = BF // 2          # 64 pairs per block
NB_FULL = FCORE // BF    # 16 blocks per core
KP = 114                 # padded pair-stack height: 50 + 14 zeros + 50

F16 = mybir.dt.float16
F32 = mybir.dt.float32


def build_program(n_blocks: int = NB_FULL) -> bass.Bass:
    nc = bass.Bass()
    nb = n_blocks

    xt_d = nc.declare_dram_parameter("xt", [nb, D, BF * L], F16, isOutput=False)
    xn_d = nc.declare_dram_parameter("xn", [nb, 2 * L, NPAIR * D], F16, isOutput=False)
    zr_d = nc.declare_dram_parameter("zeros14", [14, NPAIR * D], F16, isOutput=False)
    st_d = nc.declare_dram_parameter("st16", [D, nb * 16 * 16], F16, isOutput=False)
    id_d = nc.declare_dram_parameter("ident", [D, D], F16, isOutput=False)
    out_d = nc.declare_dram_parameter("pooledT", [nb, D, BF], F32, isOutput=True)

    with ExitStack() as ctx:
        e = ctx.enter_context
        # SBUF (per-partition KB in comments)
        xt_sb = [e(nc.sbuf_tensor(f"xt{i}", [D, BF * L], F16)) for i in range(3)]
        xn_sb = [e(nc.sbuf_tensor(f"xn{i}", [KP, NPAIR * D], F16)) for i in range(3)]
        st_sb = e(nc.sbuf_tensor("st_sb", [D, nb * 256], F16))
        id_sb = e(nc.sbuf_tensor("id_sb", [D, D], F16))
        stage_sb = [e(nc.sbuf_tensor(f"stage{i}", [16, 8 * L], F32)) for i in range(2)]
        scores_sb = [e(nc.sbuf_tensor(f"scores{i}", [BF, L], F32)) for i in range(2)]
        wexp_sb = [e(nc.sbuf_tensor(f"wexp{i}", [BF, L], F32)) for i in range(2)]
        den_sb = [e(nc.sbuf_tensor(f"den{i}", [BF, 1], F32)) for i in range(2)]
        rden_sb = [e(nc.sbuf_tensor(f"rden{i}", [BF, 1], F32)) for i in range(2)]
        wm_sb = [e(nc.sbuf_tensor(f"wm{i}", [BF, L], F16)) for i in range(2)]
        wmbd_sb = [e(nc.sbuf_tensor(f"wmbd{i}", [KP, BF], F16)) for i in range(2)]
        pooled_sb = [e(nc.sbuf_tensor(f"pooled{i}", [D, BF], F32)) for i in range(2)]
        # PSUM: one full 2KB bank per tensor so no two tensors share a bank
        ps1 = [e(nc.psum_tensor(f"ps1_{i}", [16, 512], F32)) for i in range(2)]
        pst = [e(nc.psum_tensor(f"pst{i}", [L, 1024], F16)) for i in range(2)]
        ps2 = [e(nc.psum_tensor(f"ps2_{i}", [D, 512], F32)) for i in range(2)]

        s_c1 = e(nc.semaphore("s_c1"))
        s_c2 = e(nc.semaphore("s_c2"))
        s_z = e(nc.semaphore("s_z"))
        s_ldt = [e(nc.semaphore(f"s_ldt{i}")) for i in range(3)]
        s_ldn = [[e(nc.semaphore(f"s_ldn{i}_{j}")) for j in range(4)] for i in range(3)]
        s_zn = [e(nc.semaphore(f"s_zn{i}")) for i in range(3)]
        s_mm1 = e(nc.semaphore("s_mm1"))
        s_st = e(nc.semaphore("s_st"))
        s_sc = e(nc.semaphore("s_sc"))
        s_exp = e(nc.semaphore("s_exp"))
        s_sm = e(nc.semaphore("s_sm"))
        s_tr = e(nc.semaphore("s_tr"))
        s_bd = e(nc.semaphore("s_bd"))
        s_mm2 = e(nc.semaphore("s_mm2"))
        s_pc = e(nc.semaphore("s_pc"))
        s_od = e(nc.semaphore("s_od"))
        s_rc = e(nc.semaphore("s_rc"))

        with nc.Block() as block:

            @block.sync
            def _(sync):
                sync.dma_start(st_sb[:], st_d[:]).then_inc(s_c1, 16)
                sync.dma_start(id_sb[:], id_d[:]).then_inc(s_c2, 16)
                for i in range(3):  # zero the pair-stack pad rows once
                    sync.dma_start(xn_sb[i][L:64, :], zr_d[:]).then_inc(s_zn[i], 16)
                for b in range(nb):
                    if b >= 3:  # slot reuse: block b-3's readers done
                        sync.wait_ge(s_mm1, b - 2)
                        sync.wait_ge(s_mm2, b - 2)
                    sync.dma_start(xt_sb[b % 3][:], xt_d[b]).then_inc(s_ldt[b % 3], 16)
                    # split into 8KB-per-partition packets: 16KB rows fan out
                    # to only 6 of 16 DMA engines (observed), 8KB to all 16
                    H = NPAIR * D // 2
                    sync.dma_start(
                        xn_sb[b % 3][0:L, 0:H], xn_d[b, 0:L, 0:H]
                    ).then_inc(s_ldn[b % 3][0], 16)
                    sync.dma_start(
                        xn_sb[b % 3][0:L, H:], xn_d[b, 0:L, H:]
                    ).then_inc(s_ldn[b % 3][1], 16)
                    sync.dma_start(
                        xn_sb[b % 3][64 : 64 + L, 0:H], xn_d[b, L : 2 * L, 0:H]
                    ).then_inc(s_ldn[b % 3][2], 16)
                    sync.dma_start(
                        xn_sb[b % 3][64 : 64 + L, H:], xn_d[b, L : 2 * L, H:]
                    ).then_inc(s_ldn[b % 3][3], 16)

            @block.tensor
            def _(tensor):
                tensor.wait_ge(s_c1, 16)
                tensor.wait_ge(s_c2, 16)
                for b in range(nb + 1):
                    if b < nb:
                        # ---- phase 1 (block b): 16 chunk matmuls ----
                        tensor.wait_ge(s_ldt[b % 3], 16 * (b // 3 + 1))
                        if b >= 2:
                            tensor.wait_ge(s_st, b - 1)  # ps1 slot free
                        for jj in range(16):
                            f0 = jj * 8
                            mm = nc.tensor.matmul(
                                ps1[b % 2][:, 0 : 8 * L],
                                st_sb[:, (b * 16 + jj) * 16 : (b * 16 + jj) * 16 + 16],
                                xt_sb[b % 3][:, f0 * L : f0 * L + 8 * L],
                                start=(jj == 0),
                                stop=(jj == 15),
                            )
                        mm.then_inc(s_mm1, 1)
                    if b >= 1:
                        c = b - 1
                        # ---- transpose wm(c) -> [L, BF] ----
                        tensor.wait_ge(s_sm, c + 1)
                        if c >= 2:
                            tensor.wait_ge(s_bd, c - 1)  # pst slot free
                        nc.tensor.transpose(
                            pst[c % 2][:, 0:BF], wm_sb[c % 2][:], id_sb[:]
                        ).then_inc(s_tr, 1)
                        # ---- phase 2 (block c): 64 pair matmuls ----
                        for j in range(4):
                            tensor.wait_ge(s_ldn[c % 3][j], 16 * (c // 3 + 1))
                        if c < 3:
                            tensor.wait_ge(s_zn[c % 3], 16)
                        tensor.wait_ge(s_bd, c + 1)
                        if c >= 2:
                            tensor.wait_ge(s_pc, c - 1)  # ps2 slot free
                        for p in range(NPAIR):
                            mm = nc.tensor.matmul(
                                ps2[c % 2][:, 2 * p : 2 * p + 2],
                                xn_sb[c % 3][:, p * D : (p + 1) * D],
                                wmbd_sb[c % 2][:, 2 * p : 2 * p + 2],
                                start=True,
                                stop=True,
                            )
                        mm.then_inc(s_mm2, 1)

            @block.scalar
            def _(scalar):
                for b in range(nb):
                    # stage copy: psum [16, 400] -> sbuf
                    scalar.wait_ge(s_mm1, b + 1)
                    if b >= 2:
                        scalar.wait_ge(s_sc, 16 * (b - 1))  # stage slot free
                    nc.scalar.copy(stage_sb[b % 2][:], ps1[b % 2][:, 0 : 8 * L]).then_inc(
                        s_st, 1
                    )
                    # exp + accumulate denominator
                    scalar.wait_ge(s_sc, 16 * (b + 1))
                    if b >= 2:
                        scalar.wait_ge(s_sm, b - 1)  # wexp/den slot free
                    nc.scalar.activation(
                        wexp_sb[b % 2][:],
                        scores_sb[b % 2][:],
                        mybir.ActivationFunctionType.Exp,
                        accum_out=den_sb[b % 2][:],
                    ).then_inc(s_exp, 1)

            @block.vector
            def _(vector):
                for b in range(nb):
                    # softmax normalize -> wm (f16)
                    vector.wait_ge(s_exp, b + 1)
                    nc.vector.reciprocal(rden_sb[b % 2][:], den_sb[b % 2][:]).then_inc(
                        s_rc, 1
                    )
                    vector.wait_ge(s_rc, b + 1)  # same-engine RAW (deep pipe)
                    if b >= 2:
                        vector.wait_ge(s_tr, b - 1)  # wm slot free
                    nc.vector.tensor_scalar_mul(
                        wm_sb[b % 2][:], wexp_sb[b % 2][:], rden_sb[b % 2][:]
                    ).then_inc(s_sm, 1)
                    # block-diag columns from transposed weights
                    vector.wait_ge(s_tr, b + 1)
                    if b == 0:
                        vector.wait_ge(s_z, 2)  # wmbd zero-init done
                    if b >= 2:
                        vector.wait_ge(s_mm2, b - 1)  # wmbd slot free
                    pt3 = pst[b % 2][:, 0:BF].rearrange("p (pr two) -> p pr two", two=2)
                    lo = wmbd_sb[b % 2][0:L, :].rearrange("p (pr two) -> p pr two", two=2)
                    hi = wmbd_sb[b % 2][64 : 64 + L, :].rearrange(
                        "p (pr two) -> p pr two", two=2
                    )
                    nc.vector.tensor_copy(lo[:, :, 0:1], pt3[:, :, 0:1])
                    nc.vector.tensor_copy(hi[:, :, 1:2], pt3[:, :, 1:2]).then_inc(
                        s_bd, 1
                    )
                    # evacuate pooled^T
                    vector.wait_ge(s_mm2, b + 1)
                    if b >= 2:
                        vector.wait_ge(s_od, 16 * (b - 1))  # pooled slot free
                    nc.vector.tensor_copy(
                        pooled_sb[b % 2][:], ps2[b % 2][:, 0:BF]
                    ).then_inc(s_pc, 1)

            @block.gpsimd
            def _(gpsimd):
                # one-time zero-init of both wmbd slots (off-diagonal zeros +
                # pad rows persist; per-block copies only overwrite diagonals)
                nc.gpsimd.memset(wmbd_sb[0][:], 0.0).then_inc(s_z, 1)
                nc.gpsimd.memset(wmbd_sb[1][:], 0.0).then_inc(s_z, 1)
                for b in range(nb):
                    # scatter scores: [16 slots, 8 friends, 50] -> [128, 50]
                    gpsimd.wait_ge(s_st, b + 1)
                    if b >= 1:
                        gpsimd.wait_ge(s_sc, 16 * b)  # own-sem update order
                    if b >= 2:
                        gpsimd.wait_ge(s_exp, b - 1)  # scores slot free
                    gpsimd.dma_start(
                        scores_sb[b % 2][:],
                        stage_sb[b % 2][:].rearrange("s (f l) -> s f l", l=L),
                    ).then_inc(s_sc, 16)
                    # output DMA (previous block, to keep scatter ahead)
                    if b >= 1:
                        gpsimd.wait_ge(s_pc, b)
                        gpsimd.wait_ge(s_od, 16 * (b - 1))  # own-sem update order
                        gpsimd.dma_start(
                            out_d[b - 1], pooled_sb[(b - 1) % 2][:]
                        ).then_inc(s_od, 16)
                gpsimd.wait_ge(s_pc, nb)
                gpsimd.wait_ge(s_od, 16 * (nb - 1))
                gpsimd.dma_start(out_d[nb - 1], pooled_sb[(nb - 1) % 2][:]).then_inc(
                    s_od, 16
                )
                gpsimd.wait_ge(s_od, 16 * nb)

    nc.finalize()
    return nc


def pack_inputs(friend_diff_x, self_x, friend_diff_src_mask,
                n_blocks: int = NB_FULL, ncores: int = NCORES):
    """Host-side fp16 packing + per-core slicing. Returns list of in_maps."""
    x16 = np.asarray(friend_diff_x, dtype=np.float32).astype(np.float16)
    nblk_total = ncores * n_blocks
    nrows = nblk_total * BF
    x16 = x16[:nrows]
    mk = np.asarray(friend_diff_src_mask)[:nrows, :, None]  # [rows, L, 1] bool
    # xt: [blk, d, f, l]  (unmasked)
    xt = np.ascontiguousarray(
        x16.reshape(nblk_total, BF, L, D).transpose(0, 3, 1, 2)
    ).reshape(nblk_total, D, BF * L)
    # xn: mask-premultiplied, [blk, fip, l, pair, d] padded to 114 rows:
    # rows 0:50 = fip0, rows 50:64 = zeros, rows 64:114 = fip1
    xm = np.where(mk, x16, np.float16(0))
    xm5 = xm.reshape(nblk_total, NPAIR, 2, L, D).transpose(0, 2, 3, 1, 4)
    xn = np.ascontiguousarray(xm5).reshape(nblk_total, 2 * L, NPAIR * D)
    # st16: [d, blk, jj, m] = s_{2*blk + jj//8}[d] if m == jj else 0
    s16 = np.asarray(self_x, dtype=np.float32).astype(np.float16)  # [B, D]
    n_users_total = 2 * nblk_total
    st16 = np.zeros((D, nblk_total, 16, 16), dtype=np.float16)
    for jj in range(16):
        st16[:, :, jj, jj] = s16[:n_users_total].reshape(nblk_total, 2, D)[
            :, jj // 8, :
        ].T
    st16 = st16.reshape(D, nblk_total * 256)
    ident = np.eye(D, dtype=np.float16)

    in_maps = []
    for i in range(ncores):
        in_maps.append(
            {
                "xt": xt[i * n_blocks : (i + 1) * n_blocks],
                "xn": xn[i * n_blocks : (i + 1) * n_blocks],
                "zeros14": np.zeros((14, NPAIR * D), dtype=np.float16),
                "st16": np.ascontiguousarray(
                    st16[:, i * n_blocks * 256 : (i + 1) * n_blocks * 256]
                ),
                "ident": ident,
            }
        )
    return in_maps


def unpack_output(pooledT_list, n_blocks: int = NB_FULL):
    """[ncores][n_blocks, D, BF] f32 -> [rows, D]"""
    full = np.stack(pooledT_list)  # [ncores, nb, D, BF]
    return full.transpose(0, 1, 3, 2).reshape(-1, D)


_NC_CACHE = {}


def kernel(friend_diff_x, self_x, friend_num_src, friend_num_src_tensor,
           friend_diff_src_mask, _trace=False, _trace_kwargs=None):
    assert int(friend_num_src) == FPER
    if "nc" not in _NC_CACHE:
        _NC_CACHE["nc"] = build_program(NB_FULL)
    nc = _NC_CACHE["nc"]
    in_maps = pack_inputs(friend_diff_x, self_x, friend_diff_src_mask)
    kw = {}
    if _trace:
        kw = dict(trace=True, trace_kwargs=_trace_kwargs or {})
    res = run_bass_kernel_spmd(nc, in_maps, list(range(NCORES)), **kw)
    out = unpack_output([res.results[i]["pooledT"] for i in range(NCORES)])
    kernel._last_results = res
    return out.reshape(B, FPER, D).astype(np.float32)



# revision 2
# speedup vs baseline: 1.1574x; 1.1574x over previous
"""Friend-attention pooling kernel for Trainium2 (8 NeuronCores, SPMD).

Problem (hardcoded shapes):
    friend_diff_x [16384, 50, 128] f32, self_x [256, 128] f32,
    friend_diff_src_mask [16384, 50] bool, friend_num_src == 64.
    out[b, f, :] = sum_l softmax_l(X[n] @ s[b])[l] * mask[n, l] * X[n, l, :]
    with n = b*64 + f.

Strategy: data-parallel over users across 8 cores (2048 friend rows / core,
16 blocks of 128 friends = 2 users each). SINGLE fp16 copy of X in HBM
(xt layout: [D part, pair-padded (friend, L) free]); the phase-2 layout is
produced ON-CHIP by PE transposes, halving HBM traffic vs a two-copy
scheme (profiling showed the second copy's DMA was the bottleneck: it
landed on only 10/16 SDMA engines and paced the whole kernel).

  - xt HBM: [NCH=4 chunks, D, S=4 blocks * 64 pairs * 114] f16 where each
    pair's 114 cols = [f0 history (50) | zeros (14) | f1 history (50)].
    One dma_start per 4-block chunk -> 58.4KB per-partition descriptors
    (~23GB/s/engine vs 16.9 at 12.8KB), perfectly balanced across engines.
  - phase 1 (scores): 32 matmuls/block with one-hot user stationary cols
    accumulate friend-major [16, 400] PSUM scores (moving AP strides over
    the 14-col pads) -> ACT copy -> HWDGE scatter -> [128, 50].
  - mask folds into the softmax WEIGHTS (wm = exp * 1/den * mask), not
    into X, so the single X copy serves both phases exactly.
  - 64 PE transposes/block ([128,128] slices, full-width stationary for
    fast weight load) emit the K=114 pair-stack [f0|0|f1] directly into
    PSUM; DVE/ACT alternate evacuating 8-pair groups to SBUF xnt.
  - phase 2: per pair matmul(ps2[:, 2p:2p+2], xnt pair [114,128],
    wmbd [114, 2]) - proven 25ns/MM cadence. wmbd block-diagonals are
    built zero-shift: wm is PE-transposed TWICE (out bases 0 and 64, the
    only legal sub-128 output bases) and copied into wmbd rows [0:50] /
    [64:114]; rows 50:63 stay zero from a one-time memset, which also
    annihilates the pad/garbage rows flowing through xnt.
Raw bass (manual semaphores); all cross-engine waits are standalone
wait_ge instructions; DMAs carry only their completion increment.
"""

from contextlib import ExitStack

import numpy as np

import concourse.bass as bass
from concourse import mybir
from concourse.bass_utils import run_bass_kernel_spmd

B = 256          # users
FPER = 64        # friends per user
L = 50           # history length (softmax axis)
D = 128          # embed dim
N = B * FPER     # 16384 friend rows
NCORES = 8
FCORE = N // NCORES      # 2048 friend rows per core
BF = 128                 # friends per block (= 2 users)
NPAIR = BF // 2          # 64 pairs per block
NB = FCORE // BF         # 16 blocks per core
S = 4                    # blocks per DMA chunk
NCH = NB // S            # 4 chunks per core
KP = 114                 # pair-stack height: 50 + 14 zeros + 50
PW = NPAIR * KP          # 7296 xt cols per block
SLACK = 14               # xt slot tail (last pair-transpose overreads)
GW = 8 * D               # evac group width: 8 pairs * 128 = 1024

F16 = mybir.dt.float16
F32 = mybir.dt.float32
MULT = mybir.AluOpType.mult


def build_program() -> bass.Bass:
    nc = bass.Bass()

    xt_d = nc.declare_dram_parameter("xt", [NCH, D, S * PW], F16, isOutput=False)
    st_d = nc.declare_dram_parameter("st16", [D, NB * 256], F16, isOutput=False)
    id_d = nc.declare_dram_parameter("ident", [D, D], F16, isOutput=False)
    mk_d = nc.declare_dram_parameter("maskf", [BF, NB * L], F32, isOutput=False)
    out_d = nc.declare_dram_parameter("pooledT", [2, D, 8 * BF], F32, isOutput=True)

    with ExitStack() as ctx:
        e = ctx.enter_context
        xt_sb = [e(nc.sbuf_tensor(f"xt{i}", [D, S * PW + SLACK], F16)) for i in range(2)]
        xnt_sb = [e(nc.sbuf_tensor(f"xnt{i}", [KP, NPAIR * D], F16)) for i in range(2)]
        st_sb = e(nc.sbuf_tensor("st_sb", [D, NB * 256], F16))
        id_sb = e(nc.sbuf_tensor("id_sb", [D, D], F16))
        mk_sb = e(nc.sbuf_tensor("mk_sb", [BF, NB * L], F32))
        stage_sb = [e(nc.sbuf_tensor(f"stage{i}", [16, 8 * L], F32)) for i in range(2)]
        scores_sb = [e(nc.sbuf_tensor(f"scores{i}", [BF, L], F32)) for i in range(2)]
        wexp_sb = [e(nc.sbuf_tensor(f"wexp{i}", [BF, L], F32)) for i in range(2)]
        den_sb = [e(nc.sbuf_tensor(f"den{i}", [BF, 1], F32)) for i in range(2)]
        rden_sb = [e(nc.sbuf_tensor(f"rden{i}", [BF, 1], F32)) for i in range(2)]
        wm_sb = [e(nc.sbuf_tensor(f"wm{i}", [BF, L], F16)) for i in range(2)]
        wmbd_sb = [e(nc.sbuf_tensor(f"wmbd{i}", [KP, BF], F16)) for i in range(2)]
        pooled_sb = e(nc.sbuf_tensor("pooled", [D, NB * BF], F32))
        # PSUM: one full 2KB bank per tensor
        ps1 = [e(nc.psum_tensor(f"ps1_{i}", [16, 512], F32)) for i in range(2)]
        ps2 = [e(nc.psum_tensor(f"ps2_{i}", [D, 512], F32)) for i in range(2)]
        pstw = [e(nc.psum_tensor(f"pstw{i}", [KP, 1024], F16)) for i in range(2)]
        pstx = [e(nc.psum_tensor(f"pstx{i}", [D, 1024], F16)) for i in range(2)]

        s_ld = e(nc.semaphore("s_ld"))
        s_cst = e(nc.semaphore("s_cst"))
        s_ms = e(nc.semaphore("s_ms"))
        s_mm1 = e(nc.semaphore("s_mm1"))
        s_st = e(nc.semaphore("s_st"))
        s_sc = e(nc.semaphore("s_sc"))
        s_exp = e(nc.semaphore("s_exp"))
        s_rc = e(nc.semaphore("s_rc"))
        s_sm = e(nc.semaphore("s_sm"))
        s_wt = e(nc.semaphore("s_wt"))
        s_bd = e(nc.semaphore("s_bd"))
        s_mm2 = e(nc.semaphore("s_mm2"))
        s_xt = e(nc.semaphore("s_xt"))
        s_eva = e(nc.semaphore("s_eva"))
        s_evd = e(nc.semaphore("s_evd"))
        s_pc = e(nc.semaphore("s_pc"))
        s_od = e(nc.semaphore("s_od"))

        def bank_free_wait(eng, h):
            """Wait until evac of global transpose-group h is done."""
            hb, hg = divmod(h, 8)
            if hg % 2 == 0:
                eng.wait_ge(s_evd, 4 * hb + hg // 2 + 1)
            else:
                eng.wait_ge(s_eva, 4 * hb + (hg - 1) // 2 + 1)

        with nc.Block() as block:

            @block.sync
            def _(sync):
                sync.dma_start(st_sb[:], st_d[:]).then_inc(s_cst, 16)
                sync.dma_start(id_sb[:], id_d[:]).then_inc(s_cst, 16)
                sync.dma_start(mk_sb[:], mk_d[:]).then_inc(s_cst, 16)
                for c in range(NCH):
                    if c >= 2:  # slot reuse: chunk c-2's transposes all read
                        sync.wait_ge(s_xt, 8 * S * (c - 1))
                    sync.dma_start(
                        xt_sb[c % 2][:, 0 : S * PW], xt_d[c]
                    ).then_inc(s_ld, 16)

            @block.tensor
            def _(tensor):
                tensor.wait_ge(s_cst, 32)  # st + ident loaded
                tensor.wait_ge(s_ms, 2)    # xt slot tails zeroed
                for b in range(NB + 1):
                    c = b - 1
                    sl = (b // S) % 2
                    pb = (b % S) * NPAIR
                    if b < NB:
                        # ---- phase 1 (block b): 32 strided matmuls ----
                        tensor.wait_ge(s_ld, 16 * (b // S + 1))
                        if b >= 2:
                            tensor.wait_ge(s_st, b - 1)  # ps1 slot free
                        xr = xt_sb[sl][:, 0 : S * PW].rearrange(
                            "d (p k) -> d p k", k=KP
                        )
                        o3 = ps1[b % 2][:, 0 : 8 * L].rearrange(
                            "s (f l) -> s f l", l=L
                        )
                        for jj in range(16):
                            stc = st_sb[:, (b * 16 + jj) * 16 : (b * 16 + jj) * 16 + 16]
                            for h in range(2):
                                k0 = 64 * h
                                mm = nc.tensor.matmul(
                                    o3[:, h::2, :],
                                    stc,
                                    xr[:, pb + jj * 4 : pb + (jj + 1) * 4, k0 : k0 + L],
                                    start=(jj == 0 and h == 0),
                                    stop=(jj == 15 and h == 1),
                                )
                        mm.then_inc(s_mm1, 1)

                    def xtr(g):
                        hgl = 8 * b + g - 2  # previous user of bank g%2
                        if hgl >= 0:
                            bank_free_wait(tensor, hgl)
                        for i in range(8):
                            pr = pb + g * 8 + i
                            mm = nc.tensor.transpose(
                                pstx[g % 2][:, i * D : (i + 1) * D],
                                xt_sb[sl][:, pr * KP : pr * KP + D],
                                id_sb[:],
                            )
                        mm.then_inc(s_xt, 1)

                    def wmt():
                        tensor.wait_ge(s_sm, c + 1)
                        if c >= 2:
                            tensor.wait_ge(s_bd, c - 1)  # pstw slot free
                        nc.tensor.transpose(
                            pstw[c % 2][0:L, 0:BF], wm_sb[c % 2][:], id_sb[:]
                        )
                        nc.tensor.transpose(
                            pstw[c % 2][64 : 64 + L, 0:BF], wm_sb[c % 2][:], id_sb[:]
                        ).then_inc(s_wt, 1)

                    def ph2(g):
                        if g == 0:
                            tensor.wait_ge(s_bd, c + 1)
                            if c >= 2:
                                tensor.wait_ge(s_pc, c - 1)  # ps2 slot free
                        if g % 2 == 0:
                            tensor.wait_ge(s_evd, 4 * c + g // 2 + 1)
                        else:
                            tensor.wait_ge(s_eva, 4 * c + (g + 1) // 2)
                        for i in range(8):
                            p = g * 8 + i
                            mm = nc.tensor.matmul(
                                ps2[c % 2][:, 2 * p : 2 * p + 2],
                                xnt_sb[c % 2][:, p * D : (p + 1) * D],
                                wmbd_sb[c % 2][:, 2 * p : 2 * p + 2],
                                start=True,
                                stop=True,
                            )
                        if g == 7:
                            mm.then_inc(s_mm2, 1)

                    # software-pipelined interleave of transposes (block b)
                    # with phase 2 (block b-1)
                    if b < NB:
                        xtr(0)
                        xtr(1)
                    if c >= 0:
                        wmt()
                        ph2(0)
                    for g in range(2, 8):
                        if b < NB:
                            xtr(g)
                        if c >= 0:
                            ph2(g - 1)
                    if c >= 0:
                        ph2(7)

            @block.scalar
            def _(scalar):
                for b in range(NB + 1):
                    c = b - 1
                    if c >= 0:
                        # exp + accumulate denominator (block c)
                        scalar.wait_ge(s_sc, 16 * (c + 1))
                        if c >= 2:
                            scalar.wait_ge(s_sm, c - 1)  # wexp/den slot free
                        nc.scalar.activation(
                            wexp_sb[c % 2][:],
                            scores_sb[c % 2][:],
                            mybir.ActivationFunctionType.Exp,
                            accum_out=den_sb[c % 2][:],
                        ).then_inc(s_exp, 1)
                    if b < NB:
                        # stage copy psum [16, 400] -> sbuf
                        scalar.wait_ge(s_mm1, b + 1)
                        if b >= 2:
                            scalar.wait_ge(s_sc, 16 * (b - 1))  # stage slot free
                        nc.scalar.copy(
                            stage_sb[b % 2][:], ps1[b % 2][:, 0 : 8 * L]
                        ).then_inc(s_st, 1)
                        # scatter scores [16, 8, 50] -> [128, 50] (HWDGE)
                        scalar.wait_ge(s_st, b + 1)
                        if b >= 2:
                            scalar.wait_ge(s_exp, b - 1)  # scores slot free
                        scalar.dma_start(
                            scores_sb[b % 2][:],
                            stage_sb[b % 2][:].rearrange("s (f l) -> s f l", l=L),
                        ).then_inc(s_sc, 16)
                        # evac odd transpose-groups of block b
                        for g in (1, 3, 5, 7):
                            scalar.wait_ge(s_xt, 8 * b + g + 1)
                            if g == 1 and b >= 2:
                                scalar.wait_ge(s_mm2, b - 1)  # xnt slot free
                            nc.scalar.copy(
                                xnt_sb[b % 2][:, g * GW : (g + 1) * GW],
                                pstx[g % 2][0:KP, 0:GW],
                            ).then_inc(s_eva, 1)
                    if c >= 0:
                        # evacuate pooled^T (block c)
                        scalar.wait_ge(s_mm2, c + 1)
                        nc.scalar.copy(
                            pooled_sb[:, c * BF : (c + 1) * BF], ps2[c % 2][:, 0:BF]
                        ).then_inc(s_pc, 1)
                        if c == 7:
                            scalar.wait_ge(s_pc, 8)
                            scalar.dma_start(
                                out_d[0], pooled_sb[:, 0 : 8 * BF]
                            ).then_inc(s_od, 16)
                        if c == NB - 1:
                            scalar.wait_ge(s_pc, 16)
                            scalar.dma_start(
                                out_d[1], pooled_sb[:, 8 * BF : 16 * BF]
                            ).then_inc(s_od, 16)
                            scalar.wait_ge(s_od, 32)

            @block.vector
            def _(vector):
                vector.wait_ge(s_cst, 48)  # mask loaded
                for b in range(NB + 1):
                    c = b - 1
                    if c >= 0:
                        # softmax normalize + mask fold -> wm (f16)
                        vector.wait_ge(s_exp, c + 1)
                        nc.vector.reciprocal(
                            rden_sb[c % 2][:], den_sb[c % 2][:]
                        ).then_inc(s_rc, 1)
                        vector.wait_ge(s_rc, c + 1)  # same-engine RAW (deep pipe)
                        if c >= 2:
                            vector.wait_ge(s_wt, c - 1)  # wm slot free
                        nc.vector.scalar_tensor_tensor(
                            wm_sb[c % 2][:],
                            wexp_sb[c % 2][:],
                            rden_sb[c % 2][:],
                            mk_sb[:, c * L : (c + 1) * L],
                            MULT,
                            MULT,
                        ).then_inc(s_sm, 1)
                        # wmbd block-diagonals from the two wm transposes
                        vector.wait_ge(s_wt, c + 1)
                        if c == 0:
                            vector.wait_ge(s_ms, 4)  # wmbd zero-init done
                        if c >= 2:
                            vector.wait_ge(s_mm2, c - 1)  # wmbd slot free
                        lo_d = wmbd_sb[c % 2][0:L, :].rearrange(
                            "p (pr two) -> p pr two", two=2
                        )
                        lo_s = pstw[c % 2][0:L, 0:BF].rearrange(
                            "p (pr two) -> p pr two", two=2
                        )
                        hi_d = wmbd_sb[c % 2][64 : 64 + L, :].rearrange(
                            "p (pr two) -> p pr two", two=2
                        )
                        hi_s = pstw[c % 2][64 : 64 + L, 0:BF].rearrange(
                            "p (pr two) -> p pr two", two=2
                        )
                        nc.vector.tensor_copy(lo_d[:, :, 0:1], lo_s[:, :, 0:1])
                        nc.vector.tensor_copy(hi_d[:, :, 1:2], hi_s[:, :, 1:2]).then_inc(
                            s_bd, 1
                        )
                    if b < NB:
                        # evac even transpose-groups of block b
                        for g in (0, 2, 4, 6):
                            vector.wait_ge(s_xt, 8 * b + g + 1)
                            if g == 0 and b >= 2:
                                vector.wait_ge(s_mm2, b - 1)  # xnt slot free
                            nc.vector.tensor_copy(
                                xnt_sb[b % 2][:, g * GW : (g + 1) * GW],
                                pstx[g % 2][0:KP, 0:GW],
                            ).then_inc(s_evd, 1)

            @block.gpsimd
            def _(gpsimd):
                # one-time zero-init: xt slot tails (transpose overread) and
                # wmbd (off-diagonal + pad rows persist across blocks)
                nc.gpsimd.memset(xt_sb[0][:, S * PW : S * PW + SLACK], 0.0).then_inc(
                    s_ms, 1
                )
                nc.gpsimd.memset(xt_sb[1][:, S * PW : S * PW + SLACK], 0.0).then_inc(
                    s_ms, 1
                )
                nc.gpsimd.memset(wmbd_sb[0][:], 0.0).then_inc(s_ms, 1)
                nc.gpsimd.memset(wmbd_sb[1][:], 0.0).then_inc(s_ms, 1)

    nc.finalize()
    return nc


def pack_inputs(friend_diff_x, self_x, friend_diff_src_mask):
    """Host-side fp16 packing + per-core slicing. Returns list of in_maps."""
    x16 = np.asarray(friend_diff_x, dtype=np.float32).astype(np.float16)
    xp = x16.reshape(NCORES, NB, NPAIR, 2, L, D)
    xt_full = np.zeros((NCORES, NB, NPAIR, KP, D), dtype=np.float16)
    xt_full[..., 0:L, :] = xp[:, :, :, 0, :, :]
    xt_full[..., 64 : 64 + L, :] = xp[:, :, :, 1, :, :]
    # -> [core, chunk, d, s*PW + pair*114 + k]
    xt = np.ascontiguousarray(
        xt_full.reshape(NCORES, NCH, S, PW, D).transpose(0, 1, 4, 2, 3)
    ).reshape(NCORES, NCH, D, S * PW)

    # st16[d, blk, jj, m] = s_{2*blk + jj//8}[d] if m == jj else 0
    s16 = np.asarray(self_x, dtype=np.float32).astype(np.float16)  # [B, D]
    nblk_total = NCORES * NB
    st16 = np.zeros((D, nblk_total, 16, 16), dtype=np.float16)
    for jj in range(16):
        st16[:, :, jj, jj] = s16.reshape(nblk_total, 2, D)[:, jj // 8, :].T
    st16 = st16.reshape(D, nblk_total * 256)

    mk = (
        np.asarray(friend_diff_src_mask)
        .astype(np.float32)
        .reshape(NCORES, NB, BF, L)
        .transpose(0, 2, 1, 3)
        .reshape(NCORES, BF, NB * L)
    )
    ident = np.eye(D, dtype=np.float16)

    in_maps = []
    for i in range(NCORES):
        in_maps.append(
            {
                "xt": xt[i],
                "st16": np.ascontiguousarray(
                    st16[:, i * NB * 256 : (i + 1) * NB * 256]
                ),
                "ident": ident,
                "maskf": np.ascontiguousarray(mk[i]),
            }
        )
    return in_maps


def unpack_output(pooledT_list):
    """[ncores][2, D, 8*BF] f32 -> [N, D]"""
    full = np.stack(pooledT_list)  # [ncores, 2, D, 8*BF]
    full = full.reshape(NCORES, 2, D, 8, BF).transpose(0, 1, 3, 4, 2)
    return full.reshape(N, D)


_NC_CACHE = {}


def kernel(friend_diff_x, self_x, friend_num_src, friend_num_src_tensor,
           friend_diff_src_mask, _trace=False, _trace_kwargs=None):
    assert int(friend_num_src) == FPER
    if "nc" not in _NC_CACHE:
        _NC_CACHE["nc"] = build_program()
    nc = _NC_CACHE["nc"]
    in_maps = pack_inputs(friend_diff_x, self_x, friend_diff_src_mask)
    kw = {}
    if _trace:
        kw = dict(trace=True, trace_kwargs=_trace_kwargs or {})
    res = run_bass_kernel_spmd(nc, in_maps, list(range(NCORES)), **kw)
    out = unpack_output([res.results[i]["pooledT"] for i in range(NCORES)])
    kernel._last_results = res
    return out.reshape(B, FPER, D).astype(np.float32)


# revision 11
# speedup vs baseline: 1.2554x; 1.0847x over previous
"""Friend-attention pooling kernel for Trainium2 (8 NeuronCores, SPMD).

Problem (hardcoded shapes):
    friend_diff_x [16384, 50, 128] f32, self_x [256, 128] f32,
    friend_diff_src_mask [16384, 50] bool, friend_num_src == 64.
    out[b, f, :] = sum_l softmax_l(X[n] @ s[b])[l] * mask[n, l] * X[n, l, :]
    with n = b*64 + f.

Strategy: data-parallel over users across 8 cores (2048 friend rows / core,
16 blocks of 128 friends = 2 users each). SINGLE fp16 copy of X in HBM
(xt layout: [D part, pair-padded (friend, L) free]); the phase-2 layout is
produced ON-CHIP by PE transposes, halving HBM traffic vs a two-copy
scheme (profiling showed the second copy's DMA was the bottleneck: it
landed on only 10/16 SDMA engines and paced the whole kernel).

  - xt HBM: [NCH=4 chunks, D, S=4 blocks * 64 pairs * 114] f16 where each
    pair's 114 cols = [f0 history (50) | zeros (14) | f1 history (50)].
    One dma_start per 4-block chunk -> 58.4KB per-partition descriptors
    (~23GB/s/engine vs 16.9 at 12.8KB), perfectly balanced across engines.
  - phase 1 (scores): 32 matmuls/block with one-hot user stationary cols
    accumulate friend-major [16, 400] PSUM scores (moving AP strides over
    the 14-col pads) -> ACT copy -> HWDGE scatter -> [128, 50].
  - mask folds into the softmax WEIGHTS (wm = exp * 1/den * mask), not
    into X, so the single X copy serves both phases exactly.
  - 64 PE transposes/block ([128,128] slices, full-width stationary for
    fast weight load) emit the K=114 pair-stack [f0|0|f1] directly into
    PSUM; DVE/ACT alternate evacuating 8-pair groups to SBUF xnt.
  - phase 2: per pair matmul(ps2[:, 2p:2p+2], xnt pair [114,128],
    wmbd [114, 2]) - proven 25ns/MM cadence. wmbd block-diagonals are
    built zero-shift: wm is PE-transposed TWICE (out bases 0 and 64, the
    only legal sub-128 output bases) and copied into wmbd rows [0:50] /
    [64:114]; rows 50:63 stay zero from a one-time memset, which also
    annihilates the pad/garbage rows flowing through xnt.
Raw bass (manual semaphores); all cross-engine waits are standalone
wait_ge instructions; DMAs carry only their completion increment.
"""

from contextlib import ExitStack

import numpy as np

import concourse.bass as bass
from concourse import mybir
from concourse.bass_utils import run_bass_kernel_spmd

B = 256          # users
FPER = 64        # friends per user
L = 50           # history length (softmax axis)
D = 128          # embed dim
N = B * FPER     # 16384 friend rows
NCORES = 8
FCORE = N // NCORES      # 2048 friend rows per core
BF = 128                 # friends per block (= 2 users)
NPAIR = BF // 2          # 64 pairs per block
NB = FCORE // BF         # 16 blocks per core
S = 4                    # blocks per DMA chunk
NCH = NB // S            # 4 chunks per core
KP = 114                 # pair-stack height: 50 + 14 zeros + 50
PW = NPAIR * KP          # 7296 xt cols per block
SLACK = 14               # xt slot tail (last pair-transpose overreads)
GW = 8 * D               # evac group width: 8 pairs * 128 = 1024

F16 = mybir.dt.float16
F32 = mybir.dt.float32
MULT = mybir.AluOpType.mult


def build_program() -> bass.Bass:
    nc = bass.Bass()

    xt_d = nc.declare_dram_parameter("xt", [NCH, D, S * PW], F16, isOutput=False)
    st_d = nc.declare_dram_parameter("st16", [D, NB * 256], F16, isOutput=False)
    id_d = nc.declare_dram_parameter("ident", [D, D], F16, isOutput=False)
    mk_d = nc.declare_dram_parameter("maskf", [BF, NB * L], F32, isOutput=False)
    out_d = nc.declare_dram_parameter("pooledT", [2, D, 8 * BF], F32, isOutput=True)

    with ExitStack() as ctx:
        e = ctx.enter_context
        xt_sb = [e(nc.sbuf_tensor(f"xt{i}", [D, S * PW + SLACK], F16)) for i in range(2)]
        xnt_sb = [e(nc.sbuf_tensor(f"xnt{i}", [KP, NPAIR * D], F16)) for i in range(2)]
        st_sb = e(nc.sbuf_tensor("st_sb", [D, NB * 256], F16))
        id_sb = e(nc.sbuf_tensor("id_sb", [D, D], F16))
        mk_sb = e(nc.sbuf_tensor("mk_sb", [BF, NB * L], F32))
        stage_sb = [e(nc.sbuf_tensor(f"stage{i}", [16, 8 * L], F32)) for i in range(2)]
        scores_sb = [e(nc.sbuf_tensor(f"scores{i}", [BF, L], F32)) for i in range(2)]
        wexp_sb = [e(nc.sbuf_tensor(f"wexp{i}", [BF, L], F32)) for i in range(2)]
        den_sb = [e(nc.sbuf_tensor(f"den{i}", [BF, 1], F32)) for i in range(2)]
        rden_sb = [e(nc.sbuf_tensor(f"rden{i}", [BF, 1], F32)) for i in range(2)]
        wm_sb = [e(nc.sbuf_tensor(f"wm{i}", [BF, L], F16)) for i in range(2)]
        wmbd_sb = [e(nc.sbuf_tensor(f"wmbd{i}", [KP, BF], F16)) for i in range(2)]
        pooled_sb = e(nc.sbuf_tensor("pooled", [D, NB * BF], F32))
        # PSUM: one full 2KB bank per tensor; ps1/ps2/pstw single-buffered
        # so the pair-transposes get a deep 5-bank rotation
        ps1 = e(nc.psum_tensor("ps1", [16, 512], F32))
        ps2 = e(nc.psum_tensor("ps2", [D, 512], F32))
        pstw = e(nc.psum_tensor("pstw", [KP, 1024], F16))
        NBANK = 5
        pstx = [e(nc.psum_tensor(f"pstx{i}", [D, 1024], F16)) for i in range(NBANK)]

        s_ld = e(nc.semaphore("s_ld"))
        s_cst = e(nc.semaphore("s_cst"))
        s_ms = e(nc.semaphore("s_ms"))
        s_mm1 = e(nc.semaphore("s_mm1"))
        s_st = e(nc.semaphore("s_st"))
        s_sc = e(nc.semaphore("s_sc"))
        s_exp = e(nc.semaphore("s_exp"))
        s_rc = e(nc.semaphore("s_rc"))
        s_sm = e(nc.semaphore("s_sm"))
        s_wt = e(nc.semaphore("s_wt"))
        s_bd = e(nc.semaphore("s_bd"))
        s_mm2 = e(nc.semaphore("s_mm2"))
        s_xt = e(nc.semaphore("s_xt"))
        s_eva = e(nc.semaphore("s_eva"))
        s_evd = e(nc.semaphore("s_evd"))
        s_pc = e(nc.semaphore("s_pc"))
        s_od = e(nc.semaphore("s_od"))

        EV_DVE = (0, 2, 4, 6, 7)  # evac groups handled by DVE (in this order)
        EV_ACT = (1, 3, 5)        # evac groups handled by ACT

        def ev_done_wait(eng, h):
            """Wait until evac of global transpose-group h is done."""
            hb, hg = divmod(h, 8)
            if hg in EV_DVE:
                eng.wait_ge(s_evd, len(EV_DVE) * hb + EV_DVE.index(hg) + 1)
            else:
                eng.wait_ge(s_eva, len(EV_ACT) * hb + EV_ACT.index(hg) + 1)

        with nc.Block() as block:

            @block.sync
            def _(sync):
                sync.dma_start(st_sb[:], st_d[:]).then_inc(s_cst, 16)
                sync.dma_start(id_sb[:], id_d[:]).then_inc(s_cst, 16)
                sync.dma_start(mk_sb[:], mk_d[:]).then_inc(s_cst, 16)
                for c in range(NCH):
                    if c >= 2:  # slot reuse: chunk c-2's transposes all read
                        sync.wait_ge(s_xt, 8 * S * (c - 1))
                    sync.dma_start(
                        xt_sb[c % 2][:, 0 : S * PW], xt_d[c]
                    ).then_inc(s_ld, 16)

            @block.tensor
            def _(tensor):
                tensor.wait_ge(s_cst, 32)  # st + ident loaded
                tensor.wait_ge(s_ms, 2)    # xt slot tails zeroed
                for b in range(NB + 1):
                    c = b - 1
                    sl = (b // S) % 2
                    pb = (b % S) * NPAIR
                    if b < NB:
                        # ---- phase 1 (block b): 32 strided matmuls ----
                        tensor.wait_ge(s_ld, 16 * (b // S + 1))
                        if b >= 1:
                            tensor.wait_ge(s_st, b)  # ps1 free (single bank)
                        xr = xt_sb[sl][:, 0 : S * PW].rearrange(
                            "d (p k) -> d p k", k=KP
                        )
                        o3 = ps1[:, 0 : 8 * L].rearrange(
                            "s (f l) -> s f l", l=L
                        )
                        for jj in range(16):
                            stc = st_sb[:, (b * 16 + jj) * 16 : (b * 16 + jj) * 16 + 16]
                            for h in range(2):
                                k0 = 64 * h
                                mm = nc.tensor.matmul(
                                    o3[:, h::2, :],
                                    stc,
                                    xr[:, pb + jj * 4 : pb + (jj + 1) * 4, k0 : k0 + L],
                                    start=(jj == 0 and h == 0),
                                    stop=(jj == 15 and h == 1),
                                )
                        mm.then_inc(s_mm1, 1)

                    def xtr(g):
                        gg = 8 * b + g
                        if gg >= NBANK:  # previous user of bank gg%NBANK
                            ev_done_wait(tensor, gg - NBANK)
                        for i in range(8):
                            pr = pb + g * 8 + i
                            mm = nc.tensor.transpose(
                                pstx[gg % NBANK][:, i * D : (i + 1) * D],
                                xt_sb[sl][:, pr * KP : pr * KP + D],
                                id_sb[:],
                            )
                        mm.then_inc(s_xt, 1)

                    def wmt():
                        tensor.wait_ge(s_sm, c + 1)
                        if c >= 1:
                            tensor.wait_ge(s_bd, c)  # pstw free (single bank)
                        nc.tensor.transpose(
                            pstw[0:L, 0:BF], wm_sb[c % 2][:], id_sb[:]
                        )
                        nc.tensor.transpose(
                            pstw[64 : 64 + L, 0:BF], wm_sb[c % 2][:], id_sb[:]
                        ).then_inc(s_wt, 1)

                    def ph2(g):
                        if g == 0:
                            tensor.wait_ge(s_bd, c + 1)
                            if c >= 1:
                                tensor.wait_ge(s_pc, c)  # ps2 free (single bank)
                        ev_done_wait(tensor, 8 * c + g)
                        for i in range(8):
                            p = g * 8 + i
                            mm = nc.tensor.matmul(
                                ps2[:, 2 * p : 2 * p + 2],
                                xnt_sb[c % 2][:, p * D : (p + 1) * D],
                                wmbd_sb[c % 2][:, 2 * p : 2 * p + 2],
                                start=True,
                                stop=True,
                            )
                        if g == 7:
                            mm.then_inc(s_mm2, 1)

                    # software-pipelined interleave of transposes (block b)
                    # with phase 2 (block b-1)
                    if b < NB:
                        xtr(0)
                        xtr(1)
                    if c >= 0:
                        wmt()
                        ph2(0)
                    for g in range(2, 8):
                        if b < NB:
                            xtr(g)
                        if c >= 0:
                            ph2(g - 1)
                    if c >= 0:
                        ph2(7)

            @block.scalar
            def _(scalar):
                for b in range(NB + 1):
                    c = b - 1
                    if c >= 0:
                        # exp + accumulate denominator (block c)
                        scalar.wait_ge(s_sc, 16 * (c + 1))
                        if c >= 2:
                            scalar.wait_ge(s_sm, c - 1)  # wexp/den slot free
                        nc.scalar.activation(
                            wexp_sb[c % 2][:],
                            scores_sb[c % 2][:],
                            mybir.ActivationFunctionType.Exp,
                            accum_out=den_sb[c % 2][:],
                        ).then_inc(s_exp, 1)
                    if b < NB:
                        # stage copy psum [16, 400] -> sbuf
                        scalar.wait_ge(s_mm1, b + 1)
                        if b >= 2:
                            scalar.wait_ge(s_sc, 16 * (b - 1))  # stage slot free
                        nc.scalar.copy(
                            stage_sb[b % 2][:], ps1[:, 0 : 8 * L]
                        ).then_inc(s_st, 1)
                        # scatter scores [16, 8, 50] -> [128, 50] (HWDGE)
                        scalar.wait_ge(s_st, b + 1)
                        if b >= 2:
                            scalar.wait_ge(s_exp, b - 1)  # scores slot free
                        scalar.dma_start(
                            scores_sb[b % 2][:],
                            stage_sb[b % 2][:].rearrange("s (f l) -> s f l", l=L),
                        ).then_inc(s_sc, 16)
                        # evac ACT's transpose-groups of block b
                        for g in EV_ACT:
                            scalar.wait_ge(s_xt, 8 * b + g + 1)
                            if g == EV_ACT[0] and b >= 2:
                                scalar.wait_ge(s_mm2, b - 1)  # xnt slot free
                            nc.scalar.copy(
                                xnt_sb[b % 2][:, g * GW : (g + 1) * GW],
                                pstx[(8 * b + g) % NBANK][0:KP, 0:GW],
                            ).then_inc(s_eva, 1)
                    if c >= 0:
                        # evacuate pooled^T (block c)
                        scalar.wait_ge(s_mm2, c + 1)
                        nc.scalar.copy(
                            pooled_sb[:, c * BF : (c + 1) * BF], ps2[:, 0:BF]
                        ).then_inc(s_pc, 1)
                        if c == 7:
                            scalar.wait_ge(s_pc, 8)
                            scalar.dma_start(
                                out_d[0], pooled_sb[:, 0 : 8 * BF]
                            ).then_inc(s_od, 16)
                        if c == NB - 1:
                            scalar.wait_ge(s_pc, 16)
                            scalar.dma_start(
                                out_d[1], pooled_sb[:, 8 * BF : 16 * BF]
                            ).then_inc(s_od, 16)
                            scalar.wait_ge(s_od, 32)

            @block.vector
            def _(vector):
                vector.wait_ge(s_cst, 48)  # mask loaded
                for b in range(NB + 1):
                    c = b - 1
                    if c >= 0:
                        # softmax normalize + mask fold -> wm (f16)
                        vector.wait_ge(s_exp, c + 1)
                        nc.vector.reciprocal(
                            rden_sb[c % 2][:], den_sb[c % 2][:]
                        ).then_inc(s_rc, 1)
                        vector.wait_ge(s_rc, c + 1)  # same-engine RAW (deep pipe)
                        if c >= 2:
                            vector.wait_ge(s_wt, c - 1)  # wm slot free
                        nc.vector.scalar_tensor_tensor(
                            wm_sb[c % 2][:],
                            wexp_sb[c % 2][:],
                            rden_sb[c % 2][:],
                            mk_sb[:, c * L : (c + 1) * L],
                            MULT,
                            MULT,
                        ).then_inc(s_sm, 1)
                        # wmbd block-diagonals from the two wm transposes
                        vector.wait_ge(s_wt, c + 1)
                        if c == 0:
                            vector.wait_ge(s_ms, 4)  # wmbd zero-init done
                        if c >= 2:
                            vector.wait_ge(s_mm2, c - 1)  # wmbd slot free
                        lo_d = wmbd_sb[c % 2][0:L, :].rearrange(
                            "p (pr two) -> p pr two", two=2
                        )
                        lo_s = pstw[0:L, 0:BF].rearrange(
                            "p (pr two) -> p pr two", two=2
                        )
                        hi_d = wmbd_sb[c % 2][64 : 64 + L, :].rearrange(
                            "p (pr two) -> p pr two", two=2
                        )
                        hi_s = pstw[64 : 64 + L, 0:BF].rearrange(
                            "p (pr two) -> p pr two", two=2
                        )
                        nc.vector.tensor_copy(lo_d[:, :, 0:1], lo_s[:, :, 0:1])
                        nc.vector.tensor_copy(hi_d[:, :, 1:2], hi_s[:, :, 1:2]).then_inc(
                            s_bd, 1
                        )
                    if b < NB:
                        # evac DVE's transpose-groups of block b
                        for g in EV_DVE:
                            vector.wait_ge(s_xt, 8 * b + g + 1)
                            if g == EV_DVE[0] and b >= 2:
                                vector.wait_ge(s_mm2, b - 1)  # xnt slot free
                            nc.vector.tensor_copy(
                                xnt_sb[b % 2][:, g * GW : (g + 1) * GW],
                                pstx[(8 * b + g) % NBANK][0:KP, 0:GW],
                            ).then_inc(s_evd, 1)

            @block.gpsimd
            def _(gpsimd):
                # one-time zero-init: xt slot tails (transpose overread) and
                # wmbd (off-diagonal + pad rows persist across blocks)
                nc.gpsimd.memset(xt_sb[0][:, S * PW : S * PW + SLACK], 0.0).then_inc(
                    s_ms, 1
                )
                nc.gpsimd.memset(xt_sb[1][:, S * PW : S * PW + SLACK], 0.0).then_inc(
                    s_ms, 1
                )
                nc.gpsimd.memset(wmbd_sb[0][:], 0.0).then_inc(s_ms, 1)
                nc.gpsimd.memset(wmbd_sb[1][:], 0.0).then_inc(s_ms, 1)

    nc.finalize()
    return nc


def pack_inputs(friend_diff_x, self_x, friend_diff_src_mask):
    """Host-side fp16 packing + per-core slicing. Returns list of in_maps."""
    x16 = np.asarray(friend_diff_x, dtype=np.float32).astype(np.float16)
    xp = x16.reshape(NCORES, NB, NPAIR, 2, L, D)
    xt_full = np.zeros((NCORES, NB, NPAIR, KP, D), dtype=np.float16)
    xt_full[..., 0:L, :] = xp[:, :, :, 0, :, :]
    xt_full[..., 64 : 64 + L, :] = xp[:, :, :, 1, :, :]
    # -> [core, chunk, d, s*PW + pair*114 + k]
    xt = np.ascontiguousarray(
        xt_full.reshape(NCORES, NCH, S, PW, D).transpose(0, 1, 4, 2, 3)
    ).reshape(NCORES, NCH, D, S * PW)

    # st16[d, blk, jj, m] = s_{2*blk + jj//8}[d] if m == jj else 0
    s16 = np.asarray(self_x, dtype=np.float32).astype(np.float16)  # [B, D]
    nblk_total = NCORES * NB
    st16 = np.zeros((D, nblk_total, 16, 16), dtype=np.float16)
    for jj in range(16):
        st16[:, :, jj, jj] = s16.reshape(nblk_total, 2, D)[:, jj // 8, :].T
    st16 = st16.reshape(D, nblk_total * 256)

    mk = (
        np.asarray(friend_diff_src_mask)
        .astype(np.float32)
        .reshape(NCORES, NB, BF, L)
        .transpose(0, 2, 1, 3)
        .reshape(NCORES, BF, NB * L)
    )
    ident = np.eye(D, dtype=np.float16)

    in_maps = []
    for i in range(NCORES):
        in_maps.append(
            {
                "xt": xt[i],
                "st16": np.ascontiguousarray(
                    st16[:, i * NB * 256 : (i + 1) * NB * 256]
                ),
                "ident": ident,
                "maskf": np.ascontiguousarray(mk[i]),
            }
        )
    return in_maps


def unpack_output(pooledT_list):
    """[ncores][2, D, 8*BF] f32 -> [N, D]"""
    full = np.stack(pooledT_list)  # [ncores, 2, D, 8*BF]
    full = full.reshape(NCORES, 2, D, 8, BF).transpose(0, 1, 3, 4, 2)
    return full.reshape(N, D)


_NC_CACHE = {}


def kernel(friend_diff_x, self_x, friend_num_src, friend_num_src_tensor,
           friend_diff_src_mask, _trace=False, _trace_kwargs=None):
    assert int(friend_num_src) == FPER
    if "nc" not in _NC_CACHE:
        _NC_CACHE["nc"] = build_program()
    nc = _NC_CACHE["nc"]
    in_maps = pack_inputs(friend_diff_x, self_x, friend_diff_src_mask)
    kw = {}
    if _trace:
        kw = dict(trace=True, trace_kwargs=_trace_kwargs or {})
    res = run_bass_kernel_spmd(nc, in_maps, list(range(NCORES)), **kw)
    out = unpack_output([res.results[i]["pooledT"] for i in range(NCORES)])
    kernel._last_results = res
    return out.reshape(B, FPER, D).astype(np.float32)


# revision 15
# speedup vs baseline: 1.4576x; 1.1610x over previous
"""Friend-attention pooling kernel for Trainium2 (8 NeuronCores, SPMD).

Problem (hardcoded shapes):
    friend_diff_x [16384, 50, 128] f32, self_x [256, 128] f32,
    friend_diff_src_mask [16384, 50] bool, friend_num_src == 64.
    out[b, f, :] = sum_l softmax_l(X[n] @ s[b])[l] * mask[n, l] * X[n, l, :]
    with n = b*64 + f.

Strategy: data-parallel over users across 8 cores (2048 friend rows / core,
16 blocks of 128 friends = 2 users each). SINGLE fp16 copy of X in HBM
(xt layout: [D part, pair-padded (friend, L) free]); the phase-2 layout is
produced ON-CHIP by PE transposes, halving HBM traffic vs a two-copy
scheme (profiling showed the second copy's DMA was the bottleneck: it
landed on only 10/16 SDMA engines and paced the whole kernel).

  - xt HBM: [NCH=4 chunks, D, S=4 blocks * 64 pairs * 114] f16 where each
    pair's 114 cols = [f0 history (50) | zeros (14) | f1 history (50)].
    One dma_start per 4-block chunk -> 58.4KB per-partition descriptors
    (~23GB/s/engine vs 16.9 at 12.8KB), perfectly balanced across engines.
  - phase 1 (scores): 32 matmuls/block with one-hot user stationary cols
    accumulate friend-major [16, 400] PSUM scores (moving AP strides over
    the 14-col pads) -> ACT copy -> HWDGE scatter -> [128, 50].
  - mask folds into the softmax WEIGHTS (wm = exp * 1/den * mask), not
    into X, so the single X copy serves both phases exactly.
  - 64 PE transposes/block ([128,128] slices, full-width stationary for
    fast weight load) emit the K=114 pair-stack [f0|0|f1] directly into
    PSUM; DVE/ACT alternate evacuating 8-pair groups to SBUF xnt.
  - phase 2: per pair matmul(ps2[:, 2p:2p+2], xnt pair [114,128],
    wmbd [114, 2]) - proven 25ns/MM cadence. wmbd block-diagonals are
    built zero-shift: wm is PE-transposed TWICE (out bases 0 and 64, the
    only legal sub-128 output bases) and copied into wmbd rows [0:50] /
    [64:114]; rows 50:63 stay zero from a one-time memset, which also
    annihilates the pad/garbage rows flowing through xnt.
Raw bass (manual semaphores); all cross-engine waits are standalone
wait_ge instructions; DMAs carry only their completion increment.
"""

from contextlib import ExitStack

import numpy as np

import concourse.bass as bass
from concourse import mybir
from concourse.bass_utils import run_bass_kernel_spmd

B = 256          # users
FPER = 64        # friends per user
L = 50           # history length (softmax axis)
D = 128          # embed dim
N = B * FPER     # 16384 friend rows
NCORES = 8
FCORE = N // NCORES      # 2048 friend rows per core
BF = 128                 # friends per block (= 2 users)
NPAIR = BF // 2          # 64 pairs per block
NB = FCORE // BF         # 16 blocks per core
S = 4                    # blocks per DMA chunk
NCH = NB // S            # 4 chunks per core
KP = 114                 # pair-stack height: 50 + 14 zeros + 50
PW = NPAIR * KP          # 7296 xt cols per block
SLACK = 14               # xt slot tail (last pair-transpose overreads)
GW = 8 * D               # evac group width: 8 pairs * 128 = 1024

F16 = mybir.dt.float16
F32 = mybir.dt.float32
MULT = mybir.AluOpType.mult


def build_program() -> bass.Bass:
    nc = bass.Bass()

    xt_d = nc.declare_dram_parameter("xt", [NCH, D, S * PW], F16, isOutput=False)
    st_d = nc.declare_dram_parameter("st16", [D, NB * 256], F16, isOutput=False)
    id_d = nc.declare_dram_parameter("ident", [D, D], F16, isOutput=False)
    mk_d = nc.declare_dram_parameter("maskf", [BF, NB * L], F32, isOutput=False)
    out_d = nc.declare_dram_parameter("pooledT", [2, D, 8 * BF], F32, isOutput=True)

    with ExitStack() as ctx:
        e = ctx.enter_context
        xt_sb = [e(nc.sbuf_tensor(f"xt{i}", [D, S * PW + SLACK], F16)) for i in range(2)]
        xnt_sb = [e(nc.sbuf_tensor(f"xnt{i}", [KP, NPAIR * D], F16)) for i in range(2)]
        st_sb = e(nc.sbuf_tensor("st_sb", [D, NB * 256], F16))
        id_sb = e(nc.sbuf_tensor("id_sb", [D, D], F16))
        mk_sb = e(nc.sbuf_tensor("mk_sb", [BF, NB * L], F32))
        stage_sb = [e(nc.sbuf_tensor(f"stage{i}", [16, 8 * L], F32)) for i in range(2)]
        scores_sb = [e(nc.sbuf_tensor(f"scores{i}", [BF, L], F32)) for i in range(2)]
        wexp_sb = [e(nc.sbuf_tensor(f"wexp{i}", [BF, L], F32)) for i in range(2)]
        den_sb = [e(nc.sbuf_tensor(f"den{i}", [BF, 1], F32)) for i in range(2)]
        rden_sb = [e(nc.sbuf_tensor(f"rden{i}", [BF, 1], F32)) for i in range(2)]
        wm_sb = [e(nc.sbuf_tensor(f"wm{i}", [BF, L], F16)) for i in range(2)]
        wmbd_sb = [e(nc.sbuf_tensor(f"wmbd{i}", [KP, BF], F16)) for i in range(2)]
        pooled_sb = e(nc.sbuf_tensor("pooled", [D, NB * BF], F32))
        # PSUM: one full 2KB bank per tensor; ps1/ps2/pstw single-buffered
        # so the pair-transposes get a deep 5-bank rotation
        ps1 = e(nc.psum_tensor("ps1", [16, 512], F32))
        ps2 = e(nc.psum_tensor("ps2", [D, 512], F32))
        pstw = e(nc.psum_tensor("pstw", [KP, 1024], F16))
        NBANK = 5
        pstx = [e(nc.psum_tensor(f"pstx{i}", [D, 1024], F16)) for i in range(NBANK)]

        s_ld = e(nc.semaphore("s_ld"))
        s_cst = e(nc.semaphore("s_cst"))
        s_ms = e(nc.semaphore("s_ms"))
        s_mm1 = e(nc.semaphore("s_mm1"))
        s_st = e(nc.semaphore("s_st"))
        s_sc = e(nc.semaphore("s_sc"))
        s_exp = e(nc.semaphore("s_exp"))
        s_rc = e(nc.semaphore("s_rc"))
        s_sm = e(nc.semaphore("s_sm"))
        s_wt = e(nc.semaphore("s_wt"))
        s_bd = e(nc.semaphore("s_bd"))
        s_mm2 = e(nc.semaphore("s_mm2"))
        s_xt = e(nc.semaphore("s_xt"))
        s_eva = e(nc.semaphore("s_eva"))
        s_evd = e(nc.semaphore("s_evd"))
        s_pc = e(nc.semaphore("s_pc"))
        s_od = e(nc.semaphore("s_od"))

        EV_DVE = (0, 2, 4, 6, 7)  # evac groups handled by DVE (in this order)
        EV_ACT = (1, 3, 5)        # evac groups handled by ACT

        def ev_done_wait(eng, h):
            """Wait until evac of global transpose-group h is done."""
            hb, hg = divmod(h, 8)
            if hg in EV_DVE:
                eng.wait_ge(s_evd, len(EV_DVE) * hb + EV_DVE.index(hg) + 1)
            else:
                eng.wait_ge(s_eva, len(EV_ACT) * hb + EV_ACT.index(hg) + 1)

        with nc.Block() as block:

            @block.sync
            def _(sync):
                sync.dma_start(st_sb[:], st_d[:]).then_inc(s_cst, 16)
                sync.dma_start(id_sb[:], id_d[:]).then_inc(s_cst, 16)
                sync.dma_start(mk_sb[:], mk_d[:]).then_inc(s_cst, 16)
                # chunk 0 split so block 0's slice lands ASAP
                sync.dma_start(xt_sb[0][:, 0:PW], xt_d[0, :, 0:PW]).then_inc(s_ld, 16)
                sync.dma_start(
                    xt_sb[0][:, PW : S * PW], xt_d[0, :, PW : S * PW]
                ).then_inc(s_ld, 16)
                for c in range(1, NCH):
                    if c >= 2:  # slot reuse: chunk c-2's transposes all read
                        sync.wait_ge(s_xt, 8 * S * (c - 1))
                    sync.dma_start(
                        xt_sb[c % 2][:, 0 : S * PW], xt_d[c]
                    ).then_inc(s_ld, 16)

            @block.tensor
            def _(tensor):
                tensor.wait_ge(s_cst, 32)  # st + ident loaded
                tensor.wait_ge(s_ms, 2)    # xt slot tails zeroed
                for b in range(NB + 1):
                    c = b - 1
                    sl = (b // S) % 2
                    pb = (b % S) * NPAIR
                    if b < NB:
                        # ---- phase 1 (block b): 32 strided matmuls ----
                        # s_ld counts: blk0=16, blks1-3=32, then +16/chunk
                        tensor.wait_ge(
                            s_ld, 16 if b == 0 else 16 * (b // S) + 32
                        )
                        if b >= 1:
                            tensor.wait_ge(s_st, b)  # ps1 free (single bank)
                        xr = xt_sb[sl][:, 0 : S * PW].rearrange(
                            "d (p k) -> d p k", k=KP
                        )
                        o3 = ps1[:, 0 : 8 * L].rearrange(
                            "s (f l) -> s f l", l=L
                        )
                        for jj in range(16):
                            stc = st_sb[:, (b * 16 + jj) * 16 : (b * 16 + jj) * 16 + 16]
                            for h in range(2):
                                k0 = 64 * h
                                mm = nc.tensor.matmul(
                                    o3[:, h::2, :],
                                    stc,
                                    xr[:, pb + jj * 4 : pb + (jj + 1) * 4, k0 : k0 + L],
                                    start=(jj == 0 and h == 0),
                                    stop=(jj == 15 and h == 1),
                                )
                        mm.then_inc(s_mm1, 1)

                    def xtr(g):
                        gg = 8 * b + g
                        if gg >= NBANK:  # previous user of bank gg%NBANK
                            ev_done_wait(tensor, gg - NBANK)
                        for i in range(8):
                            pr = pb + g * 8 + i
                            mm = nc.tensor.transpose(
                                pstx[gg % NBANK][:, i * D : (i + 1) * D],
                                xt_sb[sl][:, pr * KP : pr * KP + D],
                                id_sb[:],
                            )
                        mm.then_inc(s_xt, 1)

                    def wmt():
                        tensor.wait_ge(s_sm, c + 1)
                        if c >= 1:
                            tensor.wait_ge(s_bd, c)  # pstw free (single bank)
                        nc.tensor.transpose(
                            pstw[0:L, 0:BF], wm_sb[c % 2][:], id_sb[:]
                        )
                        nc.tensor.transpose(
                            pstw[64 : 64 + L, 0:BF], wm_sb[c % 2][:], id_sb[:]
                        ).then_inc(s_wt, 1)

                    def ph2(g):
                        if g == 0:
                            tensor.wait_ge(s_bd, c + 1)
                            if c >= 1:
                                tensor.wait_ge(s_pc, c)  # ps2 free (single bank)
                        ev_done_wait(tensor, 8 * c + g)
                        for i in range(8):
                            p = g * 8 + i
                            mm = nc.tensor.matmul(
                                ps2[:, 2 * p : 2 * p + 2],
                                xnt_sb[c % 2][:, p * D : (p + 1) * D],
                                wmbd_sb[c % 2][:, 2 * p : 2 * p + 2],
                                start=True,
                                stop=True,
                            )
                        if g == 7:
                            mm.then_inc(s_mm2, 1)

                    # software-pipelined interleave of transposes (block b)
                    # with phase 2 (block b-1)
                    if b < NB:
                        xtr(0)
                        xtr(1)
                    if c >= 0:
                        wmt()
                        ph2(0)
                    for g in range(2, 8):
                        if b < NB:
                            xtr(g)
                        if c >= 0:
                            ph2(g - 1)
                    if c >= 0:
                        ph2(7)

            @block.scalar
            def _(scalar):
                for b in range(NB + 1):
                    c = b - 1
                    if c >= 0:
                        # exp + accumulate denominator (block c)
                        scalar.wait_ge(s_sc, 16 * (c + 1))
                        if c >= 2:
                            scalar.wait_ge(s_sm, c - 1)  # wexp/den slot free
                        nc.scalar.activation(
                            wexp_sb[c % 2][:],
                            scores_sb[c % 2][:],
                            mybir.ActivationFunctionType.Exp,
                            accum_out=den_sb[c % 2][:],
                        ).then_inc(s_exp, 1)
                    if b < NB:
                        # stage copy psum [16, 400] -> sbuf
                        scalar.wait_ge(s_mm1, b + 1)
                        if b >= 2:
                            scalar.wait_ge(s_sc, 16 * (b - 1))  # stage slot free
                        nc.scalar.copy(
                            stage_sb[b % 2][:], ps1[:, 0 : 8 * L]
                        ).then_inc(s_st, 1)
                        # evac ACT's transpose-groups of block b
                        for g in EV_ACT:
                            scalar.wait_ge(s_xt, 8 * b + g + 1)
                            if g == EV_ACT[0] and b >= 2:
                                scalar.wait_ge(s_mm2, b - 1)  # xnt slot free
                            nc.scalar.copy(
                                xnt_sb[b % 2][:, g * GW : (g + 1) * GW],
                                pstx[(8 * b + g) % NBANK][0:KP, 0:GW],
                            ).then_inc(s_eva, 1)
                    if c >= 0:
                        # evacuate pooled^T (block c)
                        scalar.wait_ge(s_mm2, c + 1)
                        nc.scalar.copy(
                            pooled_sb[:, c * BF : (c + 1) * BF], ps2[:, 0:BF]
                        ).then_inc(s_pc, 1)
                        if c == 7:
                            scalar.wait_ge(s_pc, 8)
                            scalar.dma_start(
                                out_d[0], pooled_sb[:, 0 : 8 * BF]
                            ).then_inc(s_od, 16)
                        if c == NB - 1:
                            scalar.wait_ge(s_pc, 16)
                            scalar.dma_start(
                                out_d[1], pooled_sb[:, 8 * BF : 16 * BF]
                            ).then_inc(s_od, 16)
                            scalar.wait_ge(s_od, 32)

            @block.vector
            def _(vector):
                vector.wait_ge(s_cst, 48)  # mask loaded
                for b in range(NB + 1):
                    c = b - 1
                    if c >= 0:
                        # softmax normalize + mask fold -> wm (f16)
                        vector.wait_ge(s_exp, c + 1)
                        nc.vector.reciprocal(
                            rden_sb[c % 2][:], den_sb[c % 2][:]
                        ).then_inc(s_rc, 1)
                        vector.wait_ge(s_rc, c + 1)  # same-engine RAW (deep pipe)
                        if c >= 2:
                            vector.wait_ge(s_wt, c - 1)  # wm slot free
                        nc.vector.scalar_tensor_tensor(
                            wm_sb[c % 2][:],
                            wexp_sb[c % 2][:],
                            rden_sb[c % 2][:],
                            mk_sb[:, c * L : (c + 1) * L],
                            MULT,
                            MULT,
                        ).then_inc(s_sm, 1)
                        # wmbd block-diagonals from the two wm transposes
                        vector.wait_ge(s_wt, c + 1)
                        if c == 0:
                            vector.wait_ge(s_ms, 4)  # wmbd zero-init done
                        if c >= 2:
                            vector.wait_ge(s_mm2, c - 1)  # wmbd slot free
                        lo_d = wmbd_sb[c % 2][0:L, :].rearrange(
                            "p (pr two) -> p pr two", two=2
                        )
                        lo_s = pstw[0:L, 0:BF].rearrange(
                            "p (pr two) -> p pr two", two=2
                        )
                        hi_d = wmbd_sb[c % 2][64 : 64 + L, :].rearrange(
                            "p (pr two) -> p pr two", two=2
                        )
                        hi_s = pstw[64 : 64 + L, 0:BF].rearrange(
                            "p (pr two) -> p pr two", two=2
                        )
                        nc.vector.tensor_copy(lo_d[:, :, 0:1], lo_s[:, :, 0:1])
                        nc.vector.tensor_copy(hi_d[:, :, 1:2], hi_s[:, :, 1:2]).then_inc(
                            s_bd, 1
                        )
                    if b < NB:
                        # evac DVE's transpose-groups of block b
                        for g in EV_DVE:
                            vector.wait_ge(s_xt, 8 * b + g + 1)
                            if g == EV_DVE[0] and b >= 2:
                                vector.wait_ge(s_mm2, b - 1)  # xnt slot free
                            nc.vector.tensor_copy(
                                xnt_sb[b % 2][:, g * GW : (g + 1) * GW],
                                pstx[(8 * b + g) % NBANK][0:KP, 0:GW],
                            ).then_inc(s_evd, 1)

            @block.gpsimd
            def _(gpsimd):
                # one-time zero-init: xt slot tails (transpose overread) and
                # wmbd (off-diagonal + pad rows persist across blocks)
                nc.gpsimd.memset(xt_sb[0][:, S * PW : S * PW + SLACK], 0.0).then_inc(
                    s_ms, 1
                )
                nc.gpsimd.memset(xt_sb[1][:, S * PW : S * PW + SLACK], 0.0).then_inc(
                    s_ms, 1
                )
                nc.gpsimd.memset(wmbd_sb[0][:], 0.0).then_inc(s_ms, 1)
                nc.gpsimd.memset(wmbd_sb[1][:], 0.0).then_inc(s_ms, 1)
                for b in range(NB):
                    # scatter scores [16, 8, 50] -> [128, 50]; SWDGE merges
                    # into 16x1600B descriptors (HWDGE emits 128x200B, which
                    # crawl behind the non-preemptible 58KB chunk packets)
                    gpsimd.wait_ge(s_st, b + 1)
                    if b >= 1:
                        gpsimd.wait_ge(s_sc, 16 * b)  # own-sem update order
                    if b >= 2:
                        gpsimd.wait_ge(s_exp, b - 1)  # scores slot free
                    gpsimd.dma_start(
                        scores_sb[b % 2][:],
                        stage_sb[b % 2][:].rearrange("s (f l) -> s f l", l=L),
                    ).then_inc(s_sc, 16)

    nc.finalize()
    return nc


def pack_inputs(friend_diff_x, self_x, friend_diff_src_mask):
    """Host-side fp16 packing + per-core slicing. Returns list of in_maps."""
    x16 = np.asarray(friend_diff_x, dtype=np.float32).astype(np.float16)
    xp = x16.reshape(NCORES, NB, NPAIR, 2, L, D)
    xt_full = np.zeros((NCORES, NB, NPAIR, KP, D), dtype=np.float16)
    xt_full[..., 0:L, :] = xp[:, :, :, 0, :, :]
    xt_full[..., 64 : 64 + L, :] = xp[:, :, :, 1, :, :]
    # -> [core, chunk, d, s*PW + pair*114 + k]
    xt = np.ascontiguousarray(
        xt_full.reshape(NCORES, NCH, S, PW, D).transpose(0, 1, 4, 2, 3)
    ).reshape(NCORES, NCH, D, S * PW)

    # st16[d, blk, jj, m] = s_{2*blk + jj//8}[d] if m == jj else 0
    s16 = np.asarray(self_x, dtype=np.float32).astype(np.float16)  # [B, D]
    nblk_total = NCORES * NB
    st16 = np.zeros((D, nblk_total, 16, 16), dtype=np.float16)
    for jj in range(16):
        st16[:, :, jj, jj] = s16.reshape(nblk_total, 2, D)[:, jj // 8, :].T
    st16 = st16.reshape(D, nblk_total * 256)

    mk = (
        np.asarray(friend_diff_src_mask)
        .astype(np.float32)
        .reshape(NCORES, NB, BF, L)
        .transpose(0, 2, 1, 3)
        .reshape(NCORES, BF, NB * L)
    )
    ident = np.eye(D, dtype=np.float16)

    in_maps = []
    for i in range(NCORES):
        in_maps.append(
            {
                "xt": xt[i],
                "st16": np.ascontiguousarray(
                    st16[:, i * NB * 256 : (i + 1) * NB * 256]
                ),
                "ident": ident,
                "maskf": np.ascontiguousarray(mk[i]),
            }
        )
    return in_maps


def unpack_output(pooledT_list):
    """[ncores][2, D, 8*BF] f32 -> [N, D]"""
    full = np.stack(pooledT_list)  # [ncores, 2, D, 8*BF]
    full = full.reshape(NCORES, 2, D, 8, BF).transpose(0, 1, 3, 4, 2)
    return full.reshape(N, D)


_NC_CACHE = {}


def kernel(friend_diff_x, self_x, friend_num_src, friend_num_src_tensor,
           friend_diff_src_mask, _trace=False, _trace_kwargs=None):
    assert int(friend_num_src) == FPER
    if "nc" not in _NC_CACHE:
        _NC_CACHE["nc"] = build_program()
    nc = _NC_CACHE["nc"]
    in_maps = pack_inputs(friend_diff_x, self_x, friend_diff_src_mask)
    kw = {}
    if _trace:
        kw = dict(trace=True, trace_kwargs=_trace_kwargs or {})
    res = run_bass_kernel_spmd(nc, in_maps, list(range(NCORES)), **kw)
    out = unpack_output([res.results[i]["pooledT"] for i in range(NCORES)])
    kernel._last_results = res
    return out.reshape(B, FPER, D).astype(np.float32)
